# revision 24
# baseline (speedup 1.0000x reference)
"""Trainium2 Bass kernel for a hypernetwork-generated per-case MLP.

Math (fp32 reference):
  h = silu(o @ Wc + bc)                        [C=64, H=256]
  w = einsum('ch,lhd->lcd', h, Ww) + bw        [L=4, C, 65536]
  b = einsum('ch,lhd->lcd', h, Wb) + bb        [L=4, C, 256]
  per-case 4-layer MLP over shared x [2048, 256] with silu + skip:
    a0 = silu(x @ W0 + b0); a1 = silu(a0 @ W1 + b1)
    a2 = silu(a1 @ W2 + b2); out = (a2 + a0) @ W3 + b3
  returns [C*N, 256]

Distribution over 8 NeuronCores:
  - weight-gen tensor-sharded over the d axis of Ww (each core owns a
    contiguous 8192-wide shard, computes w[:, all 64 cases, shard]);
  - one AllToAll per layer redistributes w so core k holds full-d
    weights for its 8 cases; the l0 AllToAll is the FIRST collective
    (it absorbs the CC firmware cold-start barrier itself);
  - domain net data-parallel over cases, LAYER-OUTER over all 8 local
    cases so each later AllToAll has ~33us of compute for cover;
  - engine split: PE matmuls; ACT all silu + half the final drains;
    DVE: weight-gen psum drains, weight bias adds, skip adds, other
    half of final drains;
  - DMA queues: Ww stream + gathers + outputs on Sync; weight-gen
    staging writes on the Activation hwdge queue so the 16.8MB Ww
    stream never stalls behind a compute-dependent store.
"""

import numpy as np

import concourse.bass as bass
import concourse.mybir as mybir
import concourse.tile as tile
from concourse import bacc
from concourse.bass import ts, ds
from concourse.bass_utils import run_bass_kernel_spmd

F32 = mybir.dt.float32
F16 = mybir.dt.float16
AF = mybir.ActivationFunctionType

P = 128
NCORES = 8
C = 64          # total cases
CC = C // NCORES  # cases per core
CIN = 64        # caseNN input dim
H = 256         # caseNN hidden
HB = H // P     # h k-blocks (2)
DIN = 256       # domain feature dim (in = out = 256 for every layer)
IB = DIN // P   # 2
NL = 4          # layers
N = 2048        # samples
D = DIN * DIN   # 65536 flattened per-layer weight
DSH = D // NCORES  # 8192 per-core d shard
QD = DSH // 4   # 2048-wide quarters of the shard
_nc_cache = {}


def _build():
    nc = bacc.Bacc("TRN2", target_bir_lowering=False, debug=False, num_devices=NCORES)

    # ---- per-core external I/O ----
    xt = nc.dram_tensor("xt", [P, IB, N], F16, kind="ExternalInput").ap()
    ot = nc.dram_tensor("ot", [P, C], F16, kind="ExternalInput").ap()
    oto = nc.dram_tensor("oto", [P, CC], F16, kind="ExternalInput").ap()
    wc = nc.dram_tensor("wc", [P, H], F16, kind="ExternalInput").ap()
    bc2 = nc.dram_tensor("bc2", [P, HB], F32, kind="ExternalInput").ap()
    wws = nc.dram_tensor("wws", [NL, 4, P, HB, QD], F16, kind="ExternalInput").ap()
    wbT = nc.dram_tensor("wbT", [P, HB, NL, DIN], F16, kind="ExternalInput").ap()
    bbT = nc.dram_tensor("bbT", [P, IB, NL], F32, kind="ExternalInput").ap()
    bwT = nc.dram_tensor("bwT", [P, NL, IB, DIN], F16, kind="ExternalInput").ap()
    yt = nc.dram_tensor("yt", [CC, IB, P, N], F16, kind="ExternalOutput").ap()

    with tile.TileContext(nc) as tc:
        with (
            tc.tile_pool(name="const", bufs=1) as const,
            tc.tile_pool(name="dram", bufs=1, space="DRAM") as dram,
            tc.tile_pool(name="ww", bufs=3) as ww,
            tc.tile_pool(name="wstg", bufs=2) as wstg,
            tc.tile_pool(name="wt", bufs=2) as wtp,
            tc.tile_pool(name="a0p", bufs=1) as a0p,
            tc.tile_pool(name="a1p", bufs=1) as a1p,
            tc.tile_pool(name="tmp", bufs=3) as tmpp,
        ):
            # caseNN + weight-gen psum; closed before the domain pool opens
            ps_w_ctx = tc.tile_pool(name="ps_w", bufs=1, space="PSUM")
            ps_w = ps_w_ctx.__enter__()

            # ---- tiny consts first: keep the wgen(0) critical path clear ----
            wc_sb = const.tile([P, H], F16)
            nc.sync.dma_start(wc_sb[:], wc)
            bc_sb = const.tile([P, HB], F32)
            nc.sync.dma_start(bc_sb[:], bc2)
            ot_sb = const.tile([P, C], F16)
            nc.sync.dma_start(ot_sb[:], ot)
            oto_sb = const.tile([P, CC], F16)
            nc.sync.dma_start(oto_sb[:], oto)

            # ---- PE warm-up during the const DMAs ----
            warm = ps_w.tile([P, 512], F32, tag="pss", bufs=2, name="warm")
            for i in range(12):
                nc.tensor.matmul(warm[:, :256], lhsT=wc_sb[:, 0:P],
                                 rhs=wc_sb, start=True, stop=True)

            # ---- caseNN hidden: hT[h, c] = silu(Wc.T @ o.T + bc) ----
            hT_sb = const.tile([P, HB, C], F16)
            for kb in range(HB):
                ps = ps_w.tile([P, 512], F32, tag="pss", bufs=2, name="psh")[:, :C]
                nc.tensor.matmul(ps, lhsT=wc_sb[:, ts(kb, P)], rhs=ot_sb,
                                 start=True, stop=True)
                nc.scalar.activation(hT_sb[:, kb, :], ps, AF.Silu,
                                     bias=bc_sb[:, kb : kb + 1])

            # ---- DRAM staging for the per-layer AllToAlls ----
            w_sh = [dram.tile([C, DSH], F16, name=f"w_sh{l}") for l in range(NL)]
            w_fl = [dram.tile([C, DSH], F16, name=f"w_fl{l}") for l in range(NL)]
            # post-A2A rows: j*CC + c_loc (j = source core = d-shard index);
            # d global = i*256 + o, shard j covers i in [32j, 32j+32).
            # gather view per (ib, jr): [il, c, o] with j = 4*ib + jr
            vg = [
                w_fl[l].rearrange("(ib jr c) (il o) -> ib jr il c o",
                                  ib=IB, jr=4, c=CC, o=DIN)
                for l in range(NL)
            ]

            def wgen(l):
                """weight-gen layer l: w[c, d-shard] for all 64 cases.
                Per quarter: one [128, 1024] psum holds 4 512-chunks
                (2 col-halves x 2 free-halves), one DVE drain, one
                staging DMA on the ACT hwdge queue."""
                for q in range(4):
                    wwt = ww.tile([P, HB, QD], F16, tag="wwt", name=f"wwt{l}{q}")
                    # alternate hwdge queues so two 1MB loads stream in
                    # parallel (a single DMA only reaches ~140 GB/s)
                    ldq = nc.sync if (4 * l + q) % 2 == 0 else nc.scalar
                    ldq.dma_start(wwt[:], wws[l, q])
                    ps = ps_w.tile([P, 1024], F32, tag="psw", bufs=2,
                                   name=f"psw{l}{q}")
                    # kb-outer: one LDWEIGHTS covers 4 chunk matmuls
                    for kb in range(HB):
                        for b in range(4):  # 512-chunk index within quarter
                            h, s = divmod(b, 2)
                            nc.tensor.matmul(
                                ps[ds(h * 64, 64), ts(s, 512)],
                                lhsT=hT_sb[:, kb, :],
                                rhs=wwt[:, kb, ds(b * 512, 512)],
                                start=(kb == 0), stop=(kb == HB - 1),
                                skip_group_check=True,
                            )
                    stg = wstg.tile([P, 1024], F16, tag="wstg", name=f"stg{l}{q}")
                    nc.vector.tensor_copy(stg[:], ps)
                    # dst cols q*2048 + h*1024 + s*512 + o; rows = case
                    dstv = w_sh[l].rearrange("c (q h o2) -> c q h o2",
                                             q=4, h=2)
                    for h in range(2):
                        nc.gpsimd.dma_start(dstv[:, q, h],
                                            stg[ds(h * 64, 64), :])
                nc.gpsimd.collective_compute(
                    "AllToAll",
                    mybir.AluOpType.bypass,
                    replica_groups=[list(range(NCORES))],
                    ins=[w_sh[l].opt()],
                    outs=[w_fl[l].opt()],
                )

            # ---- braided phase A: wgen + per-layer A2A, consts in the
            # stream shadow ----
            wgen(0)

            xt_sb = const.tile([P, IB, N], F16)
            nc.sync.dma_start(xt_sb[:], xt)
            wbT_sb = const.tile([P, HB, NL, DIN], F16)
            nc.scalar.dma_start(wbT_sb[:], wbT)
            bbT_sb = const.tile([P, IB, NL], F32)
            nc.scalar.dma_start(bbT_sb[:], bbT)
            bwT_sb = const.tile([P, NL, IB, DIN], F16)
            nc.sync.dma_start(bwT_sb[:], bwT)

            wgen(1)
            wgen(2)
            wgen(3)

            # ---- caseNN for own cases: hTo + per-layer bias bO[o, ob, l, c] ----
            hTo_sb = const.tile([P, HB, CC], F16)
            for kb in range(HB):
                ps2 = ps_w.tile([P, 512], F32, tag="pss", bufs=2, name="psh2")[:, :CC]
                nc.tensor.matmul(ps2, lhsT=wc_sb[:, ts(kb, P)], rhs=oto_sb,
                                 start=True, stop=True)
                nc.scalar.activation(hTo_sb[:, kb, :], ps2, AF.Silu,
                                     bias=bc_sb[:, kb : kb + 1])
            bO_sb = const.tile([P, IB, NL, CC], F32)
            for l in range(NL):
                for ob in range(IB):
                    ps = ps_w.tile([P, 512], F32, tag="pss", bufs=2,
                                   name="psb")[:, :CC]
                    for kb in range(HB):
                        nc.tensor.matmul(
                            ps, lhsT=wbT_sb[:, kb, l, ts(ob, P)],
                            rhs=hTo_sb[:, kb, :],
                            start=(kb == 0), stop=(kb == HB - 1),
                        )
                    nc.scalar.activation(
                        bO_sb[:, ob, l, :], ps, AF.Identity,
                        bias=bbT_sb[:, ob, l : l + 1]
                    )

            # filler matmuls: bridge the PE toward the A2A(0) wait; sized to
            # undershoot (a short HAM re-warm at domain start is cheaper
            # than delaying the first domain matmul behind leftovers)
            fill = ps_w.tile([P, 512], F32, tag="pss", bufs=2, name="fill")
            for i in range(150):
                nc.tensor.matmul(fill[:, :256], lhsT=wc_sb[:, 0:P],
                                 rhs=wc_sb, start=True, stop=True)

            ps_w_ctx.__exit__(None, None, None)
            ps_y_ctx = tc.tile_pool(name="ps_y", bufs=2, space="PSUM")
            ps_y = ps_y_ctx.__enter__()

            # ---- domain net: layer-outer over all 8 local cases ----
            a0 = [None] * CC   # layer-0 output / skip accumulator / l3 input
            a1 = [None] * CC   # layer-1 output
            wt_tiles = {}

            def prep(l):
                """gather + bias-add the domain weight tiles for layer l:
                8 DMAs (split across both hwdge queues) into a per-layer
                mega-tile [P, IB, CC, DIN], then per-(c, ib) GpSimd bias
                adds (off the DVE queue so the wgen psum drains never
                head-block behind them)."""
                wt_l = wtp.tile([P, IB, CC, DIN], F16, tag="wt",
                                name=f"wt{l}")
                for jr in range(4):
                    for ib in range(IB):
                        q = nc.sync if (2 * jr + ib) % 2 == 0 else nc.scalar
                        q.dma_start(wt_l[ds(32 * jr, 32), ib],
                                    vg[l][ib, jr])
                for c in range(CC):
                    for ib in range(IB):
                        nc.gpsimd.tensor_add(wt_l[:, ib, c, :],
                                             wt_l[:, ib, c, :],
                                             bwT_sb[:, l, ib, :])
                wt_tiles[l] = wt_l

            def compute(l):
                wt_l = wt_tiles.pop(l)
                for c in range(CC):
                    if l == 0:
                        a_prev = xt_sb
                    elif l == 1:
                        a_prev = a0[c]
                    elif l == 2:
                        a_prev = a1[c]
                    else:
                        a_prev = a0[c]  # a0 += a2 done at l2
                    if l == 0:
                        a_new = a0p.tile([P, IB, N], F16, tag="a0", bufs=CC,
                                         name=f"a0_{c}")
                        a0[c] = a_new
                    elif l == 1:
                        a_new = a1p.tile([P, IB, N], F16, tag="a1", bufs=CC,
                                         name=f"a1_{c}")
                        a1[c] = a_new
                    else:
                        a_new = None  # per-ob tmp tiles
                    for ob in range(IB):
                        ps = ps_y.tile([P, 2048], F32, tag="psy",
                                       name=f"psy{c}{l}{ob}")
                        for ib in range(IB):
                            for nch in range(4):
                                nc.tensor.matmul(
                                    ps[:, ts(nch, 512)],
                                    lhsT=wt_l[:, ib, c, ts(ob, P)],
                                    rhs=a_prev[:, ib, ds(nch * 512, 512)],
                                    start=(ib == 0), stop=(ib == IB - 1),
                                )
                        bias = bO_sb[:, ob, l, c : c + 1]
                        if l < 2:
                            nc.scalar.activation(a_new[:, ob, :], ps, AF.Silu,
                                                 bias=bias)
                        elif l == 2:
                            t = tmpp.tile([P, N], F16, tag="tmp",
                                          name=f"t2_{c}{ob}")
                            nc.scalar.activation(t[:], ps, AF.Silu, bias=bias)
                            # skip: a0 += a2 (feeds the final layer)
                            nc.vector.tensor_add(a0[c][:, ob, :],
                                                 a0[c][:, ob, :], t[:])
                        else:
                            t = tmpp.tile([P, N], F16, tag="tmp",
                                          name=f"t3_{c}{ob}")
                            nc.vector.tensor_scalar_add(t[:], ps, bias)
                            nc.sync.dma_start(yt[c, ob], t[:])

            # de-prioritize prep emission so the static scheduler never
            # hoists its (A2A-gated) ops above phase-A staging work
            with tc.high_priority(offset=-10_000_000):
                prep(0)
            for l in range(NL):
                if l + 1 < NL:
                    with tc.high_priority(offset=-10_000_000):
                        prep(l + 1)
                compute(l)
            ps_y_ctx.__exit__(None, None, None)

    nc.compile()
    return nc


def _prep_inputs(x, o, Wc, bc, Ww, bw, Wb, bb):
    x = np.asarray(x, np.float32)
    o = np.asarray(o, np.float32)
    Wc = np.asarray(Wc, np.float32)
    bc = np.asarray(bc, np.float32)
    Ww = np.asarray(Ww, np.float32)
    bw = np.asarray(bw, np.float32)
    Wb = np.asarray(Wb, np.float32)
    bb = np.asarray(bb, np.float32)

    xt = np.ascontiguousarray(x.T.reshape(IB, P, N).transpose(1, 0, 2)).astype(np.float16)
    otf = np.zeros((P, C), np.float16)
    otf[:CIN, :] = o.T
    wcp = np.zeros((P, H), np.float16)
    wcp[:CIN, :] = Wc
    bc2 = np.ascontiguousarray(bc.reshape(HB, P).T)
    wbT = np.ascontiguousarray(Wb.reshape(NL, HB, P, DIN).transpose(2, 1, 0, 3)).astype(np.float16)
    bbT = np.ascontiguousarray(bb.reshape(NL, IB, P).transpose(2, 1, 0))
    bwT = np.ascontiguousarray(bw.reshape(NL, IB, P, DIN).transpose(2, 0, 1, 3)).astype(np.float16)

    in_maps = []
    for k in range(NCORES):
        in_maps.append(
            {
                "xt": xt,
                "ot": otf,
                "oto": np.ascontiguousarray(otf[:, k * CC : (k + 1) * CC]),
                "wc": wcp,
                "bc2": bc2,
                "wws": np.ascontiguousarray(
                    Ww[:, :, k * DSH : (k + 1) * DSH]
                    .reshape(NL, HB, P, 4, QD)
                    .transpose(0, 3, 2, 1, 4)
                ).astype(np.float16),
                "wbT": wbT,
                "bbT": bbT,
                "bwT": bwT,
            }
        )
    return in_maps


def _run(inputs, trace=False):
    if "nc" not in _nc_cache:
        _nc_cache["nc"] = _build()
    nc = _nc_cache["nc"]
    in_maps = _prep_inputs(**inputs)
    res = run_bass_kernel_spmd(
        nc, in_maps, core_ids=list(range(NCORES)), trace=trace
    )
    # yt per core: [CC, IB, P, N] f16 -> [CC, N, IB*P] case-major
    parts = []
    for k in range(NCORES):
        ytk = res.results[k]["yt"].astype(np.float32)
        parts.append(ytk.transpose(0, 3, 1, 2).reshape(CC, N, DIN))
    out = np.concatenate(parts, axis=0).reshape(C * N, DIN)
    return out, res


def kernel(**inputs):
    out, _ = _run(inputs, trace=False)
    return out


# revision 26
# speedup vs baseline: 1.0741x; 1.0741x over previous
"""Trainium2 Bass kernel for a hypernetwork-generated per-case MLP.

Math (fp32 reference):
  h = silu(o @ Wc + bc)                        [C=64, H=256]
  w = einsum('ch,lhd->lcd', h, Ww) + bw        [L=4, C, 65536]
  b = einsum('ch,lhd->lcd', h, Wb) + bb        [L=4, C, 256]
  per-case 4-layer MLP over shared x [2048, 256] with silu + skip:
    a0 = silu(x @ W0 + b0); a1 = silu(a0 @ W1 + b1)
    a2 = silu(a1 @ W2 + b2); out = (a2 + a0) @ W3 + b3
  returns [C*N, 256]

Distribution over 8 NeuronCores:
  - weight-gen tensor-sharded over the d axis of Ww (each core owns a
    contiguous 8192-wide shard, computes w[:, all 64 cases, shard]);
  - one AllToAll per layer redistributes w so core k holds full-d
    weights for its 8 cases; the l0 AllToAll is the FIRST collective
    (it absorbs the CC firmware cold-start barrier itself);
  - domain net data-parallel over cases, LAYER-OUTER over all 8 local
    cases so each later AllToAll has ~33us of compute for cover;
  - engine split: PE matmuls; ACT all silu + half the final drains;
    DVE: weight-gen psum drains, weight bias adds, skip adds, other
    half of final drains;
  - DMA queues: Ww stream + gathers + outputs on Sync; weight-gen
    staging writes on the Activation hwdge queue so the 16.8MB Ww
    stream never stalls behind a compute-dependent store.
"""

import numpy as np

import concourse.bass as bass
import concourse.mybir as mybir
import concourse.tile as tile
from concourse import bacc
from concourse.bass import ts, ds
from concourse.bass_utils import run_bass_kernel_spmd

F32 = mybir.dt.float32
F16 = mybir.dt.float16
AF = mybir.ActivationFunctionType

P = 128
NCORES = 8
C = 64          # total cases
CC = C // NCORES  # cases per core
CIN = 64        # caseNN input dim
H = 256         # caseNN hidden
HB = H // P     # h k-blocks (2)
DIN = 256       # domain feature dim (in = out = 256 for every layer)
IB = DIN // P   # 2
NL = 4          # layers
N = 2048        # samples
D = DIN * DIN   # 65536 flattened per-layer weight
DSH = D // NCORES  # 8192 per-core d shard
QD = DSH // 4   # 2048-wide quarters of the shard
_nc_cache = {}


def _build():
    nc = bacc.Bacc("TRN2", target_bir_lowering=False, debug=False, num_devices=NCORES)

    # ---- per-core external I/O ----
    xt = nc.dram_tensor("xt", [P, IB, N], F16, kind="ExternalInput").ap()
    ot = nc.dram_tensor("ot", [P, C], F16, kind="ExternalInput").ap()
    oto = nc.dram_tensor("oto", [P, CC], F16, kind="ExternalInput").ap()
    wc = nc.dram_tensor("wc", [P, H], F16, kind="ExternalInput").ap()
    bc2 = nc.dram_tensor("bc2", [P, HB], F32, kind="ExternalInput").ap()
    wws = nc.dram_tensor("wws", [NL, 4, P, HB, QD], F16, kind="ExternalInput").ap()
    wbT = nc.dram_tensor("wbT", [P, HB, NL, DIN], F16, kind="ExternalInput").ap()
    bbT = nc.dram_tensor("bbT", [P, IB, NL], F32, kind="ExternalInput").ap()
    bwT = nc.dram_tensor("bwT", [P, NL, IB, DIN], F16, kind="ExternalInput").ap()
    yt = nc.dram_tensor("yt", [CC, IB, P, N], F16, kind="ExternalOutput").ap()

    with tile.TileContext(nc) as tc:
        with (
            tc.tile_pool(name="const", bufs=1) as const,
            tc.tile_pool(name="dram", bufs=1, space="DRAM") as dram,
            tc.tile_pool(name="ww", bufs=3) as ww,
            tc.tile_pool(name="wstg", bufs=2) as wstg,
            tc.tile_pool(name="wt", bufs=2) as wtp,
            tc.tile_pool(name="a0p", bufs=1) as a0p,
            tc.tile_pool(name="a1p", bufs=1) as a1p,
            tc.tile_pool(name="tmp", bufs=3) as tmpp,
        ):
            # caseNN + weight-gen psum; closed before the domain pool opens
            ps_w_ctx = tc.tile_pool(name="ps_w", bufs=1, space="PSUM")
            ps_w = ps_w_ctx.__enter__()

            # ---- tiny consts first: keep the wgen(0) critical path clear ----
            wc_sb = const.tile([P, H], F16)
            nc.sync.dma_start(wc_sb[:], wc)
            bc_sb = const.tile([P, HB], F32)
            nc.sync.dma_start(bc_sb[:], bc2)
            ot_sb = const.tile([P, C], F16)
            nc.sync.dma_start(ot_sb[:], ot)
            oto_sb = const.tile([P, CC], F16)
            nc.sync.dma_start(oto_sb[:], oto)

            # ---- PE warm-up during the const DMAs ----
            warm = ps_w.tile([P, 512], F32, tag="pss", bufs=2, name="warm")
            for i in range(12):
                nc.tensor.matmul(warm[:, :256], lhsT=wc_sb[:, 0:P],
                                 rhs=wc_sb, start=True, stop=True)

            # ---- caseNN hidden: hT[h, c] = silu(Wc.T @ o.T + bc) ----
            hT_sb = const.tile([P, HB, C], F16)
            for kb in range(HB):
                ps = ps_w.tile([P, 512], F32, tag="pss", bufs=2, name="psh")[:, :C]
                nc.tensor.matmul(ps, lhsT=wc_sb[:, ts(kb, P)], rhs=ot_sb,
                                 start=True, stop=True)
                nc.scalar.activation(hT_sb[:, kb, :], ps, AF.Silu,
                                     bias=bc_sb[:, kb : kb + 1])

            # ---- DRAM staging for the per-layer AllToAlls ----
            w_sh = [dram.tile([C, DSH], F16, name=f"w_sh{l}") for l in range(NL)]
            w_fl = [dram.tile([C, DSH], F16, name=f"w_fl{l}") for l in range(NL)]
            # post-A2A rows: j*CC + c_loc (j = source core = d-shard index);
            # d global = i*256 + o, shard j covers i in [32j, 32j+32).
            # gather view per (ib, jr): [il, c, o] with j = 4*ib + jr
            vg = [
                w_fl[l].rearrange("(ib jr c) (il o) -> ib jr il c o",
                                  ib=IB, jr=4, c=CC, o=DIN)
                for l in range(NL)
            ]

            def wgen(l):
                """weight-gen layer l: w[c, d-shard] for all 64 cases.
                Per quarter: one [128, 1024] psum holds 4 512-chunks
                (2 col-halves x 2 free-halves), one DVE drain, one
                staging DMA on the ACT hwdge queue."""
                for q in range(4):
                    wwt = ww.tile([P, HB, QD], F16, tag="wwt", name=f"wwt{l}{q}")
                    # alternate hwdge queues so two 1MB loads stream in
                    # parallel (a single DMA only reaches ~140 GB/s)
                    ldq = nc.sync if (4 * l + q) % 2 == 0 else nc.scalar
                    ldq.dma_start(wwt[:], wws[l, q])
                    ps = ps_w.tile([P, 1024], F32, tag="psw", bufs=2,
                                   name=f"psw{l}{q}")
                    # kb-outer: one LDWEIGHTS covers 4 chunk matmuls
                    for kb in range(HB):
                        for b in range(4):  # 512-chunk index within quarter
                            h, s = divmod(b, 2)
                            nc.tensor.matmul(
                                ps[ds(h * 64, 64), ts(s, 512)],
                                lhsT=hT_sb[:, kb, :],
                                rhs=wwt[:, kb, ds(b * 512, 512)],
                                start=(kb == 0), stop=(kb == HB - 1),
                                skip_group_check=True,
                            )
                    stg = wstg.tile([P, 1024], F16, tag="wstg", name=f"stg{l}{q}")
                    nc.vector.tensor_copy(stg[:], ps)
                    # dst cols q*2048 + h*1024 + s*512 + o; rows = case
                    dstv = w_sh[l].rearrange("c (q h o2) -> c q h o2",
                                             q=4, h=2)
                    for h in range(2):
                        nc.scalar.dma_start(dstv[:, q, h],
                                            stg[ds(h * 64, 64), :])
                nc.gpsimd.collective_compute(
                    "AllToAll",
                    mybir.AluOpType.bypass,
                    replica_groups=[list(range(NCORES))],
                    ins=[w_sh[l].opt()],
                    outs=[w_fl[l].opt()],
                )

            # ---- braided phase A: wgen + per-layer A2A, consts in the
            # stream shadow ----
            wgen(0)

            xt_sb = const.tile([P, IB, N], F16)
            nc.sync.dma_start(xt_sb[:], xt)
            wbT_sb = const.tile([P, HB, NL, DIN], F16)
            nc.scalar.dma_start(wbT_sb[:], wbT)
            bbT_sb = const.tile([P, IB, NL], F32)
            nc.scalar.dma_start(bbT_sb[:], bbT)
            bwT_sb = const.tile([P, NL, IB, DIN], F16)
            nc.sync.dma_start(bwT_sb[:], bwT)

            wgen(1)
            wgen(2)
            wgen(3)

            # ---- caseNN for own cases: hTo + per-layer bias bO[o, ob, l, c] ----
            hTo_sb = const.tile([P, HB, CC], F16)
            for kb in range(HB):
                ps2 = ps_w.tile([P, 512], F32, tag="pss", bufs=2, name="psh2")[:, :CC]
                nc.tensor.matmul(ps2, lhsT=wc_sb[:, ts(kb, P)], rhs=oto_sb,
                                 start=True, stop=True)
                nc.scalar.activation(hTo_sb[:, kb, :], ps2, AF.Silu,
                                     bias=bc_sb[:, kb : kb + 1])
            bO_sb = const.tile([P, IB, NL, CC], F32)
            for l in range(NL):
                for ob in range(IB):
                    ps = ps_w.tile([P, 512], F32, tag="pss", bufs=2,
                                   name="psb")[:, :CC]
                    for kb in range(HB):
                        nc.tensor.matmul(
                            ps, lhsT=wbT_sb[:, kb, l, ts(ob, P)],
                            rhs=hTo_sb[:, kb, :],
                            start=(kb == 0), stop=(kb == HB - 1),
                        )
                    nc.scalar.activation(
                        bO_sb[:, ob, l, :], ps, AF.Identity,
                        bias=bbT_sb[:, ob, l : l + 1]
                    )

            # filler matmuls: bridge the PE toward the A2A(0) wait; sized to
            # undershoot (a short HAM re-warm at domain start is cheaper
            # than delaying the first domain matmul behind leftovers)
            fill = ps_w.tile([P, 512], F32, tag="pss", bufs=2, name="fill")
            for i in range(150):
                nc.tensor.matmul(fill[:, :256], lhsT=wc_sb[:, 0:P],
                                 rhs=wc_sb, start=True, stop=True)

            ps_w_ctx.__exit__(None, None, None)
            ps_y_ctx = tc.tile_pool(name="ps_y", bufs=2, space="PSUM")
            ps_y = ps_y_ctx.__enter__()

            # ---- domain net: layer-outer over all 8 local cases ----
            a0 = [None] * CC   # layer-0 output / skip accumulator / l3 input
            a1 = [None] * CC   # layer-1 output
            wt_tiles = {}

            def prep(l):
                """gather + bias-add the domain weight tiles for layer l:
                8 DMAs (split across both hwdge queues) into a per-layer
                mega-tile [P, IB, CC, DIN], then per-(c, ib) GpSimd bias
                adds (off the DVE queue so the wgen psum drains never
                head-block behind them)."""
                wt_l = wtp.tile([P, IB, CC, DIN], F16, tag="wt",
                                name=f"wt{l}")
                for jr in range(4):
                    for ib in range(IB):
                        q = nc.sync if (2 * jr + ib) % 2 == 0 else nc.scalar
                        q.dma_start(wt_l[ds(32 * jr, 32), ib],
                                    vg[l][ib, jr])
                for c in range(CC):
                    for ib in range(IB):
                        nc.gpsimd.tensor_add(wt_l[:, ib, c, :],
                                             wt_l[:, ib, c, :],
                                             bwT_sb[:, l, ib, :])
                wt_tiles[l] = wt_l

            def compute(l):
                wt_l = wt_tiles.pop(l)
                for c in range(CC):
                    if l == 0:
                        a_prev = xt_sb
                    elif l == 1:
                        a_prev = a0[c]
                    elif l == 2:
                        a_prev = a1[c]
                    else:
                        a_prev = a0[c]  # a0 += a2 done at l2
                    if l == 0:
                        a_new = a0p.tile([P, IB, N], F16, tag="a0", bufs=CC,
                                         name=f"a0_{c}")
                        a0[c] = a_new
                    elif l == 1:
                        a_new = a1p.tile([P, IB, N], F16, tag="a1", bufs=CC,
                                         name=f"a1_{c}")
                        a1[c] = a_new
                    else:
                        a_new = None  # per-ob tmp tiles
                    for ob in range(IB):
                        ps = ps_y.tile([P, 2048], F32, tag="psy",
                                       name=f"psy{c}{l}{ob}")
                        for ib in range(IB):
                            for nch in range(4):
                                nc.tensor.matmul(
                                    ps[:, ts(nch, 512)],
                                    lhsT=wt_l[:, ib, c, ts(ob, P)],
                                    rhs=a_prev[:, ib, ds(nch * 512, 512)],
                                    start=(ib == 0), stop=(ib == IB - 1),
                                )
                        bias = bO_sb[:, ob, l, c : c + 1]
                        if l < 2:
                            nc.scalar.activation(a_new[:, ob, :], ps, AF.Silu,
                                                 bias=bias)
                        elif l == 2:
                            t = tmpp.tile([P, N], F16, tag="tmp",
                                          name=f"t2_{c}{ob}")
                            nc.scalar.activation(t[:], ps, AF.Silu, bias=bias)
                            # skip: a0 += a2 (feeds the final layer)
                            nc.vector.tensor_add(a0[c][:, ob, :],
                                                 a0[c][:, ob, :], t[:])
                        else:
                            t = tmpp.tile([P, N], F16, tag="tmp",
                                          name=f"t3_{c}{ob}")
                            nc.vector.tensor_scalar_add(t[:], ps, bias)
                            nc.sync.dma_start(yt[c, ob], t[:])

            # de-prioritize prep emission so the static scheduler never
            # hoists its (A2A-gated) ops above phase-A staging work
            with tc.high_priority(offset=-10_000_000):
                prep(0)
            # HAM re-warm: these matmuls READ the first gathered weight
            # tile, so they can only start once the l0 gather lands --
            # exactly the window right before the first domain matmul.
            # (Values are throwaway; the psum is never consumed.)
            wt_l0 = wt_tiles[0]
            for i in range(14):
                nc.tensor.matmul(fill[:, :256], lhsT=wc_sb[:, 0:P],
                                 rhs=wt_l0[:, 0, 0, :], start=True, stop=True)
            for l in range(NL):
                if l + 1 < NL:
                    with tc.high_priority(offset=-10_000_000):
                        prep(l + 1)
                compute(l)
            ps_y_ctx.__exit__(None, None, None)

    nc.compile()
    return nc


def _prep_inputs(x, o, Wc, bc, Ww, bw, Wb, bb):
    x = np.asarray(x, np.float32)
    o = np.asarray(o, np.float32)
    Wc = np.asarray(Wc, np.float32)
    bc = np.asarray(bc, np.float32)
    Ww = np.asarray(Ww, np.float32)
    bw = np.asarray(bw, np.float32)
    Wb = np.asarray(Wb, np.float32)
    bb = np.asarray(bb, np.float32)

    xt = np.ascontiguousarray(x.T.reshape(IB, P, N).transpose(1, 0, 2)).astype(np.float16)
    otf = np.zeros((P, C), np.float16)
    otf[:CIN, :] = o.T
    wcp = np.zeros((P, H), np.float16)
    wcp[:CIN, :] = Wc
    bc2 = np.ascontiguousarray(bc.reshape(HB, P).T)
    wbT = np.ascontiguousarray(Wb.reshape(NL, HB, P, DIN).transpose(2, 1, 0, 3)).astype(np.float16)
    bbT = np.ascontiguousarray(bb.reshape(NL, IB, P).transpose(2, 1, 0))
    bwT = np.ascontiguousarray(bw.reshape(NL, IB, P, DIN).transpose(2, 0, 1, 3)).astype(np.float16)

    in_maps = []
    for k in range(NCORES):
        in_maps.append(
            {
                "xt": xt,
                "ot": otf,
                "oto": np.ascontiguousarray(otf[:, k * CC : (k + 1) * CC]),
                "wc": wcp,
                "bc2": bc2,
                "wws": np.ascontiguousarray(
                    Ww[:, :, k * DSH : (k + 1) * DSH]
                    .reshape(NL, HB, P, 4, QD)
                    .transpose(0, 3, 2, 1, 4)
                ).astype(np.float16),
                "wbT": wbT,
                "bbT": bbT,
                "bwT": bwT,
            }
        )
    return in_maps


def _run(inputs, trace=False):
    if "nc" not in _nc_cache:
        _nc_cache["nc"] = _build()
    nc = _nc_cache["nc"]
    in_maps = _prep_inputs(**inputs)
    res = run_bass_kernel_spmd(
        nc, in_maps, core_ids=list(range(NCORES)), trace=trace
    )
    # yt per core: [CC, IB, P, N] f16 -> [CC, N, IB*P] case-major
    parts = []
    for k in range(NCORES):
        ytk = res.results[k]["yt"].astype(np.float32)
        parts.append(ytk.transpose(0, 3, 1, 2).reshape(CC, N, DIN))
    out = np.concatenate(parts, axis=0).reshape(C * N, DIN)
    return out, res


def kernel(**inputs):
    out, _ = _run(inputs, trace=False)
    return out
